# revision 20
# baseline (speedup 1.0000x reference)
"""Boundary rendering module for Trainium2 (8 NeuronCores).

Computes, for x of shape (2, 4, 64, 256, 256) f32:
    mn/mx  = per-channel global min/max
    binary = ((x - mn) / (mx - mn)) > 0.5     [== (x - mn) > 0.5*(mx - mn)]
    dilated = 3x3x3 binary dilation of binary (SAME padding)
    out    = dilated - binary

Sharding: H (=256) split into 8 chunks of 32 rows, one per NeuronCore.
Each core receives its 32 rows plus one halo row on each side (global
edges padded with -1e30 so the halo mask is 0).  On-core layout puts
(B, D) = 128 on the SBUF partition axis; (C, H, W) live on the free axis.

Min/max needs a global reduction: per-core partials are reduced across
partitions with gpsimd.partition_all_reduce, then across the 8 cores with
an AllReduce(max) collective over the pair (mx, -mn).

The 3x3x3 dilation is computed as a count:
    count[p, h, w] = sum_{dd,dw in {-1,0,1}} sum_q bandA[p,q] * mH[q, h, w+dw]
where mH is the H-dilated binary mask (2 vector max ops) and bandA is the
(b,d)-banded 0/1 matrix (D-axis window).  The W shifts are plain +-1 column
offsets into a zero-padded mask buffer, accumulated in PSUM by TensorE.
A final accumulating matmul adds -16 * binary, so
    out = 1  iff  count >= 1 and binary == 0  iff  psum >= 1
which a single saturated sigmoid activation turns into exact {0.0, 1.0}.
"""

import os
import sys

import numpy as np

for _p in ("/opt/trn_rl_repo", "/root/.axon_site/_ro/trn_rl_repo"):
    if os.path.isdir(_p) and _p not in sys.path:
        sys.path.insert(0, _p)

import ml_dtypes

B, C, D, H, W = 2, 4, 64, 256, 256
NCORES = 8
HS = H // NCORES  # 32 own rows per core
HPAD = np.float32(-1e30)  # halo pad at global H edges -> mask 0

MHW = 258  # mH row width: 256 data cols + 2 zero pad cols
MHLEN = 33 * MHW + 2  # 33 rows (1 pad + 32 data) + slack for dw=+1 AP views

_CACHE = {}


def _band_matrices():
    bd = np.arange(128)
    b = bd // D
    d = bd % D
    A = (b[:, None] == b[None, :]) & (np.abs(d[:, None] - d[None, :]) <= 1)
    A = A.astype(ml_dtypes.bfloat16)
    negI = (-16.0 * np.eye(128)).astype(ml_dtypes.bfloat16)
    return A, negI


def _build():
    import concourse.bass as bass
    import concourse.bacc as bacc
    import concourse.mybir as mybir
    import concourse.tile as tile
    from contextlib import ExitStack

    f32 = mybir.dt.float32
    bf16 = mybir.dt.bfloat16
    Alu = mybir.AluOpType

    nc = bacc.Bacc(
        "TRN2",
        target_bir_lowering=False,
        debug=False,
        num_devices=NCORES,
    )

    xs = nc.dram_tensor("xs", [B, C, D, HS + 2, W], f32, kind="ExternalInput")
    out = nc.dram_tensor("out", [B, C, D, HS, W], f32, kind="ExternalOutput")
    ccin = nc.dram_tensor("ccin", [1, 8], f32)
    ccout = nc.dram_tensor("ccout", [1, 8], f32)
    trd = nc.dram_tensor("trd", [128, 8], f32)  # partial transpose bounce
    A_np, negI_np = _band_matrices()
    bandA_d = nc.inline_tensor(A_np, name="bandA")
    negI_d = nc.inline_tensor(negI_np, name="negI")

    # partition axis = (b, d) = 128; DRAM-side APs keep b and d as separate
    # leading dims (DMA pairs elements in iteration order, b-major then d,
    # matching partition index p = b*64 + d).
    xsa = xs.ap()
    outa = out.ap()

    with ExitStack() as ctx:
        tc = ctx.enter_context(tile.TileContext(nc))
        pers = ctx.enter_context(tc.tile_pool(name="pers", bufs=1))
        outp = ctx.enter_context(tc.tile_pool(name="outp", bufs=2))
        psump = ctx.enter_context(tc.tile_pool(name="psum", bufs=2, space="PSUM"))

        x_own = pers.tile([128, C, HS, W], f32)  # 128 KiB / partition
        x_halo = pers.tile([128, C, 2, W], f32)  # 8 KiB
        binm = pers.tile([128, HS + 2, W], bf16)  # 17 KiB  {0,1}
        mH = pers.tile([128, MHLEN], bf16)  # ~16.7 KiB  H-dilated mask
        pmax = pers.tile([128, 16], f32)
        pmin = pers.tile([128, 16], f32)
        red8 = pers.tile([128, 8], f32)  # [mx(4) | -mn(4)] local
        red8b = pers.tile([128, 8], f32)  # single-writer copy of red8
        r1 = pers.tile([128, 1024], f32)  # transposed partials (partition 0)
        s1 = pers.tile([128, 8], f32)  # reduced over partitions (partition 0)
        gb8 = pers.tile([128, 8], f32)  # after cross-core all-reduce
        mnv = pers.tile([128, 4], f32)  # mn per channel
        h4 = pers.tile([128, 4], f32)  # 0.5*(mx-mn) per channel
        At = pers.tile([128, 128], bf16)
        Nt = pers.tile([128, 128], bf16)
        sel_bias = pers.tile([128, 1], f32)
        nc.vector.memset(sel_bias[:, :], -100.0)

        nc.gpsimd.dma_start(out=At[:, :], in_=bandA_d.ap())
        nc.gpsimd.dma_start(out=Nt[:, :], in_=negI_d.ap())
        nc.vector.memset(mH[:, :], 0.0)  # zero pads once; data rows rewritten

        # ---- pass 1: load x, per-core per-channel min/max ----
        for c in range(C):
            for k in range(4):
                nc.sync.dma_start(
                    out=x_own[:, c, 8 * k : 8 * k + 8, :],
                    in_=xsa[:, c, :, 1 + 8 * k : 9 + 8 * k, :],
                )
            nc.gpsimd.dma_start(out=x_halo[:, c, 0, :], in_=xsa[:, c, :, 0, :])
            nc.gpsimd.dma_start(out=x_halo[:, c, 1, :], in_=xsa[:, c, :, HS + 1, :])

        for c in range(C):
            for k in range(4):
                chunk = x_own[:, c, 8 * k : 8 * k + 8, :]
                nc.vector.tensor_reduce(
                    out=pmax[:, 4 * c + k : 4 * c + k + 1],
                    in_=chunk,
                    axis=mybir.AxisListType.XY,
                    op=Alu.max,
                )
                nc.vector.tensor_reduce(
                    out=pmin[:, 4 * c + k : 4 * c + k + 1],
                    in_=chunk,
                    axis=mybir.AxisListType.XY,
                    op=Alu.min,
                )
        for c in range(C):
            nc.vector.tensor_reduce(
                out=red8[:, c : c + 1],
                in_=pmax[:, 4 * c : 4 * c + 4],
                axis=mybir.AxisListType.X,
                op=Alu.max,
            )
            nc.vector.tensor_reduce(
                out=red8[:, 4 + c : 5 + c],
                in_=pmin[:, 4 * c : 4 * c + 4],
                axis=mybir.AxisListType.X,
                op=Alu.min,
            )
        # negate the mins so one max-allreduce handles both
        nc.vector.tensor_scalar_mul(red8[:, 4:8], red8[:, 4:8], -1.0)
        # single-writer copy so the DMA below needs only one sync wait
        nc.vector.tensor_copy(red8b[:, :], red8[:, :])

        # cross-partition max via a DRAM-bounce transpose: [128 parts, 8] ->
        # [1 part, 8, 128], then a free-axis reduce on partition 0.
        nc.gpsimd.dma_start(out=trd.ap(), in_=red8b[:, :])
        trd_t = bass.AP(tensor=trd.ap().tensor, offset=0, ap=[[1, 8], [8, 128]])
        nc.gpsimd.dma_start(out=r1[0:1, :], in_=trd_t)
        nc.vector.tensor_reduce(
            out=s1[0:1, 0:8],
            in_=r1[0:1, :].rearrange("p (j q) -> p j q", q=128),
            axis=mybir.AxisListType.X,
            op=Alu.max,
        )
        nc.gpsimd.dma_start(out=ccin.ap(), in_=s1[0:1, 0:8])
        nc.gpsimd.collective_compute(
            "AllReduce",
            Alu.max,
            replica_groups=[list(range(NCORES))],
            ins=[ccin.ap().opt()],
            outs=[ccout.ap().opt()],
        )
        # broadcast the 8 reduced values to all 128 partitions in one DMA
        bcast = bass.AP(tensor=ccout.ap().tensor, offset=0, ap=[[0, 128], [1, 8]])
        nc.gpsimd.dma_start(out=gb8[:, :], in_=bcast)

        nc.vector.tensor_scalar_mul(mnv[:, :], gb8[:, 4:8], -1.0)
        nc.vector.tensor_add(h4[:, :], gb8[:, 0:4], gb8[:, 4:8])
        nc.vector.tensor_scalar_mul(h4[:, :], h4[:, :], 0.5)

        # ---- pass 2: mask, dilate, boundary ----
        # mH data row r (1..32) = H-dilated mask of own row r-1, at flat
        # offset r*MHW, cols 0..255; col 256/257 and row 0 stay zero.
        mHd = mH[:, MHW : MHW + 32 * MHW].rearrange("p (r z) -> p r z", z=MHW)[
            :, :, 0:W
        ]
        for c in range(C):
            nc.vector.tensor_scalar(
                out=binm[:, 1 : HS + 1, :],
                in0=x_own[:, c, :, :],
                scalar1=mnv[:, c : c + 1],
                scalar2=h4[:, c : c + 1],
                op0=Alu.subtract,
                op1=Alu.is_gt,
            )
            nc.vector.tensor_scalar(
                out=binm[:, 0, :],
                in0=x_halo[:, c, 0, :],
                scalar1=mnv[:, c : c + 1],
                scalar2=h4[:, c : c + 1],
                op0=Alu.subtract,
                op1=Alu.is_gt,
            )
            nc.vector.tensor_scalar(
                out=binm[:, HS + 1, :],
                in0=x_halo[:, c, 1, :],
                scalar1=mnv[:, c : c + 1],
                scalar2=h4[:, c : c + 1],
                op0=Alu.subtract,
                op1=Alu.is_gt,
            )
            # H dilation (rows of binm are halo+own+halo)
            nc.vector.tensor_tensor(
                out=mHd,
                in0=binm[:, 0:HS, :],
                in1=binm[:, 2 : HS + 2, :],
                op=Alu.max,
            )
            nc.vector.tensor_tensor(
                out=mHd,
                in0=mHd,
                in1=binm[:, 1 : HS + 1, :],
                op=Alu.max,
            )
            for t in range(4):  # 8 own rows per psum tile
                ps = psump.tile([128, 2048], mybir.dt.float32)
                for s in range(4):  # one PSUM bank = 2 rows = 512
                    R = 8 * t + 2 * s
                    pslice = ps[:, 512 * s : 512 * s + 512]
                    for j, dw in enumerate((-1, 0, 1)):
                        off = (R + 1) * MHW + dw
                        rhs = mH[:, off : off + 2 * MHW].rearrange(
                            "p (r z) -> p r z", z=MHW
                        )[:, :, 0:W]
                        nc.tensor.matmul(
                            pslice,
                            At[:, :],
                            rhs,
                            start=(j == 0),
                            stop=False,
                        )
                    nc.tensor.matmul(
                        pslice,
                        Nt[:, :],
                        binm[:, 1 + R : 3 + R, :],
                        start=False,
                        stop=True,
                    )
                ob = outp.tile([128, 2048], mybir.dt.float32)
                nc.scalar.activation(
                    out=ob[:, :],
                    in_=ps[:, :],
                    func=mybir.ActivationFunctionType.Sigmoid,
                    bias=sel_bias[:, :],
                    scale=200.0,
                )
                nc.sync.dma_start(
                    out=outa[:, c, :, 8 * t : 8 * t + 8, :],
                    in_=ob.rearrange("p (r w) -> p r w", w=W),
                )

    nc.compile()
    return nc


def _get_nc():
    if "nc" not in _CACHE:
        _CACHE["nc"] = _build()
    return _CACHE["nc"]


def _make_in_maps(x: np.ndarray):
    in_maps = []
    for k in range(NCORES):
        xs = np.empty((B, C, D, HS + 2, W), np.float32)
        lo = k * HS
        xs[:, :, :, 1 : HS + 1, :] = x[:, :, :, lo : lo + HS, :]
        if k > 0:
            xs[:, :, :, 0, :] = x[:, :, :, lo - 1, :]
        else:
            xs[:, :, :, 0, :] = HPAD
        if k < NCORES - 1:
            xs[:, :, :, HS + 1, :] = x[:, :, :, lo + HS, :]
        else:
            xs[:, :, :, HS + 1, :] = HPAD
        in_maps.append({"xs": xs})
    return in_maps


def kernel(x: np.ndarray) -> np.ndarray:
    from concourse.bass_utils import run_bass_kernel_spmd

    x = np.ascontiguousarray(np.asarray(x), dtype=np.float32)
    assert x.shape == (B, C, D, H, W)

    in_maps = _make_in_maps(x)
    nc = _get_nc()
    res = run_bass_kernel_spmd(nc, in_maps, core_ids=list(range(NCORES)))
    pieces = [res.results[k]["out"] for k in range(NCORES)]
    return np.concatenate(pieces, axis=3)


if __name__ == "__main__":
    x = np.random.randn(B, C, D, H, W).astype(np.float32)
    y = kernel(x)
    print(y.shape, y.dtype, y.sum())


# revision 26
# speedup vs baseline: 3.0011x; 3.0011x over previous
"""Boundary rendering module for Trainium2 (8 NeuronCores).

Computes, for x of shape (2, 4, 64, 256, 256) f32:
    mn/mx  = per-channel global min/max
    binary = ((x - mn) / (mx - mn)) > 0.5     [== (x - mn) > 0.5*(mx - mn)]
    dilated = 3x3x3 binary dilation of binary (SAME padding)
    out    = dilated - binary

Sharding: H (=256) split into 8 chunks of 32 rows, one per NeuronCore.
Each core receives its 32 rows plus one halo row on each side (global
edges padded with -1e30 so the halo mask is 0).  On-core layout puts
(B, D) = 128 on the SBUF partition axis; (C, H, W) live on the free axis.

Per-channel min/max: per-partition partials on DVE, transposed across
partitions with a tiny PE matmul against an identity, reduced on DVE,
then an 8-core AllReduce(max) collective over the pair (mx, -mn), and
broadcast back to 128 partitions with a rank-1 PE matmul.

The 3x3x3 dilation is computed as a count:
    count[p, h, w] = sum_{dw in {-1,0,1}} sum_q bandA[p,q] * mH[q, h, w+dw]
where mH is the H-dilated binary mask (2 vector max ops) and bandA is the
(b,d)-banded 0/1 matrix (D-axis window).  The W shifts are plain +-1 column
offsets into a zero-padded mask buffer, accumulated in PSUM by TensorE.
A final accumulating matmul adds -16 * binary, so
    psum >= 1  iff  count >= 1 and binary == 0
which a single saturated sigmoid activation turns into exact {0.0, 1.0}.
"""

import os
import sys

import numpy as np

for _p in ("/opt/trn_rl_repo", "/root/.axon_site/_ro/trn_rl_repo"):
    if os.path.isdir(_p) and _p not in sys.path:
        sys.path.insert(0, _p)

import ml_dtypes

B, C, D, H, W = 2, 4, 64, 256, 256
NCORES = 8
HS = H // NCORES  # 32 own rows per core
HA = HS + 2  # rows incl halo
HPAD = np.float32(-1e30)  # halo pad at global H edges -> mask 0

MHW = 258  # mH row width: 256 data cols + 2 zero pad cols
MHLEN = 33 * MHW + 2  # 33 rows (1 pad + 32 data) + slack for dw=+1 AP views

_CACHE = {}


def _consts():
    bd = np.arange(128)
    b = bd // D
    d = bd % D
    A = (b[:, None] == b[None, :]) & (np.abs(d[:, None] - d[None, :]) <= 1)
    A = A.astype(ml_dtypes.bfloat16)
    negI = (-16.0 * np.eye(128)).astype(ml_dtypes.bfloat16)
    I128 = np.eye(128, dtype=np.float32)
    return A, negI, I128


def _build(reps: int = 1, use_collective: bool = True):
    import concourse.bass as bass
    import concourse.bacc as bacc
    import concourse.mybir as mybir
    import concourse.tile as tile
    from contextlib import ExitStack

    f32 = mybir.dt.float32
    bf16 = mybir.dt.bfloat16
    Alu = mybir.AluOpType

    nc = bacc.Bacc(
        "TRN2",
        target_bir_lowering=False,
        debug=False,
        num_devices=NCORES,
    )

    xs = nc.dram_tensor("xs", [B, C, D, HA, W], f32, kind="ExternalInput")
    out = nc.dram_tensor("out", [B, C, D, HS, W], f32, kind="ExternalOutput")
    ccin = nc.dram_tensor("ccin", [8, 1], f32)
    ccout = nc.dram_tensor("ccout", [8, 1], f32)
    A_np, negI_np, I_np = _consts()
    bandA_d = nc.inline_tensor(A_np, name="bandA")
    negI_d = nc.inline_tensor(negI_np, name="negI")
    ident_d = nc.inline_tensor(I_np, name="ident")

    # partition axis = (b, d) = 128; DRAM-side APs keep b and d as separate
    # leading dims (DMA pairs elements in iteration order, b-major then d,
    # matching partition index p = b*64 + d).
    xsa = xs.ap()
    outa = out.ap()

    with ExitStack() as ctx:
        tc = ctx.enter_context(tile.TileContext(nc))
        pers = ctx.enter_context(tc.tile_pool(name="pers", bufs=1))
        psump = ctx.enter_context(tc.tile_pool(name="psum", bufs=2, space="PSUM"))

        x_all = pers.tile([128, C, HA, W], f32)  # 136 KiB / partition
        binm = pers.tile([128, HA, W], bf16)  # 17 KiB  {0,1}
        mH = pers.tile([128, MHLEN], bf16)  # ~16.7 KiB  H-dilated mask
        stag = pers.tile([128, 4096], f32)  # out staging (16 rows)
        pmax = pers.tile([128, 16], f32)
        pmin = pers.tile([128, 16], f32)
        red8 = pers.tile([128, 8], f32)  # [mx(4) | -mn(4)] local
        s8 = pers.tile([128, 1], f32)  # per-partition reduced (parts 0..7)
        s1v = pers.tile([128, 8], f32)  # allreduced values (partition 0)
        gv8 = pers.tile([128, 8], f32)  # broadcast [mx | -mn] on all parts
        mnv = pers.tile([128, 4], f32)  # mn per channel
        h4 = pers.tile([128, 4], f32)  # 0.5*(mx-mn) per channel
        At = pers.tile([128, 128], bf16)
        Nt = pers.tile([128, 128], bf16)
        It = pers.tile([128, 128], f32)
        ones1 = pers.tile([128, 128], f32)  # row 0 used as all-ones lhsT
        sel_bias = pers.tile([128, 1], f32)

        nc.vector.memset(sel_bias[:, :], -100.0)
        nc.vector.memset(ones1[:, :], 1.0)
        nc.gpsimd.dma_start(out=At[:, :], in_=bandA_d.ap())
        nc.gpsimd.dma_start(out=Nt[:, :], in_=negI_d.ap())
        nc.gpsimd.dma_start(out=It[:, :], in_=ident_d.ap())
        nc.vector.memset(mH[:, :], 0.0)  # zero pads once; data rows rewritten

        for _rep in range(reps):
            _one_pass(
                nc, bass, mybir, Alu, tc, psump, xsa, outa, ccin, ccout,
                x_all, binm, mH, stag, pmax, pmin, red8, s8, s1v, gv8,
                mnv, h4, At, Nt, It, ones1, sel_bias, use_collective,
            )

    nc.compile()
    return nc


def _one_pass(
    nc, bass, mybir, Alu, tc, psump, xsa, outa, ccin, ccout,
    x_all, binm, mH, stag, pmax, pmin, red8, s8, s1v, gv8,
    mnv, h4, At, Nt, It, ones1, sel_bias, use_collective,
):
    f32 = mybir.dt.float32

    # ---- pass 1: load x (34 rows incl halo, one DMA per channel), then
    # per-core per-channel min/max over the 32 own rows ----
    for c in range(C):
        nc.sync.dma_start(out=x_all[:, c, :, :], in_=xsa[:, c, :, :, :])

    for c in range(C):
        for k in range(4):
            chunk = x_all[:, c, 1 + 8 * k : 9 + 8 * k, :]
            nc.vector.tensor_reduce(
                out=pmax[:, 4 * c + k : 4 * c + k + 1],
                in_=chunk,
                axis=mybir.AxisListType.XY,
                op=Alu.max,
            )
            nc.vector.tensor_reduce(
                out=pmin[:, 4 * c + k : 4 * c + k + 1],
                in_=chunk,
                axis=mybir.AxisListType.XY,
                op=Alu.min,
            )
    for c in range(C):
        nc.vector.tensor_reduce(
            out=red8[:, c : c + 1],
            in_=pmax[:, 4 * c : 4 * c + 4],
            axis=mybir.AxisListType.X,
            op=Alu.max,
        )
        nc.vector.tensor_reduce(
            out=red8[:, 4 + c : 5 + c],
            in_=pmin[:, 4 * c : 4 * c + 4],
            axis=mybir.AxisListType.X,
            op=Alu.min,
        )
    # negate the mins so one max-allreduce handles both
    nc.vector.tensor_scalar_mul(red8[:, 4:8], red8[:, 4:8], -1.0)

    # cross-partition max: transpose red8 [128p, 8] -> psum [8p, 128] with a
    # PE matmul against the identity, then free-axis reduce on DVE.
    pst = psump.tile([128, 2048], f32, tag="ps")
    nc.tensor.matmul(pst[0:8, 0:128], red8[:, :], It[:, :], start=True, stop=True)
    nc.vector.tensor_reduce(
        out=s8[0:8, 0:1],
        in_=pst[0:8, 0:128],
        axis=mybir.AxisListType.X,
        op=Alu.max,
    )
    nc.sync.dma_start(out=ccin.ap(), in_=s8[0:8, 0:1])
    if use_collective:
        nc.gpsimd.collective_compute(
            "AllReduce",
            Alu.max,
            replica_groups=[list(range(NCORES))],
            ins=[ccin.ap().opt()],
            outs=[ccout.ap().opt()],
        )
    else:
        nc.gpsimd.dma_start(out=ccout.ap(), in_=ccin.ap())
    # load the 8 reduced values onto partition 0, broadcast to all 128
    # partitions with a rank-1 matmul (ones[1,128]^T @ vals[1,8]).
    nc.sync.dma_start(out=s1v[0:1, :], in_=ccout.ap().rearrange("a b -> b a"))
    psb = psump.tile([128, 2048], f32, tag="ps")
    nc.tensor.matmul(psb[:, 0:8], ones1[0:1, :], s1v[0:1, :], start=True, stop=True)
    nc.vector.tensor_copy(gv8[:, :], psb[:, 0:8])
    nc.vector.tensor_scalar_mul(mnv[:, :], gv8[:, 4:8], -1.0)
    nc.vector.tensor_add(h4[:, :], gv8[:, 0:4], gv8[:, 4:8])
    nc.vector.tensor_scalar_mul(h4[:, :], h4[:, :], 0.5)

    # ---- pass 2: mask, dilate, boundary ----
    # mH data row r (1..32) = H-dilated mask of own row r-1, at flat
    # offset r*MHW, cols 0..255; col 256/257 and row 0 stay zero.
    mHd = mH[:, MHW : MHW + 32 * MHW].rearrange("p (r z) -> p r z", z=MHW)[
        :, :, 0:W
    ]
    for c in range(C):
        # binary mask for all 34 rows (halo included) in one fused op:
        # (x - mn) > h
        nc.vector.tensor_scalar(
            out=binm[:, :, :],
            in0=x_all[:, c, :, :],
            scalar1=mnv[:, c : c + 1],
            scalar2=h4[:, c : c + 1],
            op0=Alu.subtract,
            op1=Alu.is_gt,
        )
        # H dilation
        nc.vector.tensor_tensor(
            out=mHd,
            in0=binm[:, 0:HS, :],
            in1=binm[:, 2 : HS + 2, :],
            op=Alu.max,
        )
        nc.vector.tensor_tensor(
            out=mHd,
            in0=mHd,
            in1=binm[:, 1 : HS + 1, :],
            op=Alu.max,
        )
        for t in range(2):  # 16 own rows per staging buffer
            ps = psump.tile([128, 2048], f32, tag="ps")
            ps2 = psump.tile([128, 2048], f32, tag="ps")
            for half, pst_ in ((0, ps), (1, ps2)):
                for s in range(4):  # one PSUM bank = 2 rows = 512
                    R = 16 * t + 8 * half + 2 * s
                    pslice = pst_[:, 512 * s : 512 * s + 512]
                    for j, dw in enumerate((-1, 0, 1)):
                        off = (R + 1) * MHW + dw
                        rhs = mH[:, off : off + 2 * MHW].rearrange(
                            "p (r z) -> p r z", z=MHW
                        )[:, :, 0:W]
                        nc.tensor.matmul(
                            pslice,
                            At[:, :],
                            rhs,
                            start=(j == 0),
                            stop=False,
                        )
                    nc.tensor.matmul(
                        pslice,
                        Nt[:, :],
                        binm[:, 1 + R : 3 + R, :],
                        start=False,
                        stop=True,
                    )
                nc.scalar.activation(
                    out=stag[:, 2048 * half : 2048 * half + 2048],
                    in_=pst_[:, :],
                    func=mybir.ActivationFunctionType.Sigmoid,
                    bias=sel_bias[:, :],
                    scale=200.0,
                )
            nc.sync.dma_start(
                out=outa[:, c, :, 16 * t : 16 * t + 16, :],
                in_=stag.rearrange("p (r w) -> p r w", w=W),
            )


def _get_nc():
    if "nc" not in _CACHE:
        _CACHE["nc"] = _build()
    return _CACHE["nc"]


def _make_in_maps(x: np.ndarray):
    in_maps = []
    for k in range(NCORES):
        xs = np.empty((B, C, D, HA, W), np.float32)
        lo = k * HS
        xs[:, :, :, 1 : HS + 1, :] = x[:, :, :, lo : lo + HS, :]
        if k > 0:
            xs[:, :, :, 0, :] = x[:, :, :, lo - 1, :]
        else:
            xs[:, :, :, 0, :] = HPAD
        if k < NCORES - 1:
            xs[:, :, :, HS + 1, :] = x[:, :, :, lo + HS, :]
        else:
            xs[:, :, :, HS + 1, :] = HPAD
        in_maps.append({"xs": xs})
    return in_maps


def kernel(x: np.ndarray) -> np.ndarray:
    from concourse.bass_utils import run_bass_kernel_spmd

    x = np.ascontiguousarray(np.asarray(x), dtype=np.float32)
    assert x.shape == (B, C, D, H, W)

    in_maps = _make_in_maps(x)
    nc = _get_nc()
    res = run_bass_kernel_spmd(nc, in_maps, core_ids=list(range(NCORES)))
    pieces = [res.results[k]["out"] for k in range(NCORES)]
    return np.concatenate(pieces, axis=3)


if __name__ == "__main__":
    x = np.random.randn(B, C, D, H, W).astype(np.float32)
    y = kernel(x)
    print(y.shape, y.dtype, y.sum())


# revision 28
# speedup vs baseline: 81.0443x; 27.0051x over previous
"""Boundary rendering module for Trainium2 (8 NeuronCores).

Computes, for x of shape (2, 4, 64, 256, 256) f32:
    mn/mx  = per-channel global min/max
    binary = ((x - mn) / (mx - mn)) > 0.5     [== (x - mn) > 0.5*(mx - mn)]
    dilated = 3x3x3 binary dilation of binary (SAME padding)
    out    = dilated - binary

Sharding: H (=256) split into 8 chunks of 32 rows, one per NeuronCore.
Each core receives its 32 rows plus one halo row on each side (global
edges padded with -1e30 so the halo mask is 0).  On-core layout puts
(B, D) = 128 on the SBUF partition axis; (C, H, W) live on the free axis.

Per-channel min/max: per-partition partials on DVE, transposed across
partitions with a tiny PE matmul against an identity, reduced on DVE,
then an 8-core AllReduce(max) collective over the pair (mx, -mn), and
broadcast back to 128 partitions with a rank-1 PE matmul.

The 3x3x3 dilation is computed as a count:
    count[p, h, w] = sum_{dw in {-1,0,1}} sum_q bandA[p,q] * mH[q, h, w+dw]
where mH is the H-dilated binary mask (2 vector max ops) and bandA is the
(b,d)-banded 0/1 matrix (D-axis window).  The W shifts are plain +-1 column
offsets into a zero-padded mask buffer, accumulated in PSUM by TensorE.
A final accumulating matmul adds -16 * binary, so
    psum >= 1  iff  count >= 1 and binary == 0
which a single saturated sigmoid activation turns into exact {0.0, 1.0}.
"""

import os
import sys

import numpy as np

for _p in ("/opt/trn_rl_repo", "/root/.axon_site/_ro/trn_rl_repo"):
    if os.path.isdir(_p) and _p not in sys.path:
        sys.path.insert(0, _p)

import ml_dtypes

B, C, D, H, W = 2, 4, 64, 256, 256
NCORES = 8
HS = H // NCORES  # 32 own rows per core
HA = HS + 2  # rows incl halo
HPAD = np.float32(-1e30)  # halo pad at global H edges -> mask 0

MHW = 258  # mH row width: 256 data cols + 2 zero pad cols
MHLEN = 33 * MHW + 2  # 33 rows (1 pad + 32 data) + slack for dw=+1 AP views

_CACHE = {}


def _consts():
    bd = np.arange(128)
    b = bd // D
    d = bd % D
    A = (b[:, None] == b[None, :]) & (np.abs(d[:, None] - d[None, :]) <= 1)
    A = A.astype(ml_dtypes.bfloat16)
    negI = (-16.0 * np.eye(128)).astype(ml_dtypes.bfloat16)
    I128 = np.eye(128, dtype=np.float32)
    return A, negI, I128


def _build(reps: int = 1, phase: str = "B"):
    import concourse.bass as bass
    import concourse.bacc as bacc
    import concourse.mybir as mybir
    import concourse.tile as tile
    from contextlib import ExitStack

    f32 = mybir.dt.float32
    bf16 = mybir.dt.bfloat16
    Alu = mybir.AluOpType

    nc = bacc.Bacc(
        "TRN2",
        target_bir_lowering=False,
        debug=False,
        num_devices=NCORES,
    )

    xs = nc.dram_tensor("xs", [B, C, D, HA, W], f32, kind="ExternalInput")
    if phase == "A":
        pmm = nc.dram_tensor("pmm", [8, 1], f32, kind="ExternalOutput")
        out = pm64 = None
    else:
        pm64 = nc.dram_tensor("pm64", [NCORES, 8], f32, kind="ExternalInput")
        out = nc.dram_tensor("out", [B, C, D, HS, W], f32, kind="ExternalOutput")
        pmm = None
    A_np, negI_np, I_np = _consts()
    bandA_d = nc.inline_tensor(A_np, name="bandA")
    negI_d = nc.inline_tensor(negI_np, name="negI")
    ident_d = nc.inline_tensor(I_np, name="ident")

    # partition axis = (b, d) = 128; DRAM-side APs keep b and d as separate
    # leading dims (DMA pairs elements in iteration order, b-major then d,
    # matching partition index p = b*64 + d).
    xsa = xs.ap()
    outa = out.ap() if out is not None else None

    with ExitStack() as ctx:
        tc = ctx.enter_context(tile.TileContext(nc))
        pers = ctx.enter_context(tc.tile_pool(name="pers", bufs=1))
        psump = ctx.enter_context(tc.tile_pool(name="psum", bufs=2, space="PSUM"))

        x_all = pers.tile([128, C, HA, W], f32)  # 136 KiB / partition
        binm = pers.tile([128, HA, W], bf16)  # 17 KiB  {0,1}
        mH = pers.tile([128, MHLEN], bf16)  # ~16.7 KiB  H-dilated mask
        stag = pers.tile([128, 4096], f32)  # out staging (16 rows)
        pmax = pers.tile([128, 16], f32)
        pmin = pers.tile([128, 16], f32)
        red8 = pers.tile([128, 8], f32)  # [mx(4) | -mn(4)] local
        s8 = pers.tile([128, 1], f32)  # per-partition reduced (parts 0..7)
        s1v = pers.tile([128, 72], f32)  # gathered partials + reduced vals
        gv8 = pers.tile([128, 8], f32)  # broadcast [mx | -mn] on all parts
        mnv = pers.tile([128, 4], f32)  # mn per channel
        h4 = pers.tile([128, 4], f32)  # 0.5*(mx-mn) per channel
        At = pers.tile([128, 128], bf16)
        Nt = pers.tile([128, 128], bf16)
        It = pers.tile([128, 128], f32)
        ones1 = pers.tile([128, 128], f32)  # row 0 used as all-ones lhsT
        sel_bias = pers.tile([128, 1], f32)

        nc.vector.memset(sel_bias[:, :], -100.0)
        nc.vector.memset(ones1[:, :], 1.0)
        nc.gpsimd.dma_start(out=At[:, :], in_=bandA_d.ap())
        nc.gpsimd.dma_start(out=Nt[:, :], in_=negI_d.ap())
        nc.gpsimd.dma_start(out=It[:, :], in_=ident_d.ap())
        nc.vector.memset(mH[:, :], 0.0)  # zero pads once; data rows rewritten

        for _rep in range(reps):
            if phase == "A":
                _pass_a(
                    nc, mybir, Alu, psump, xsa, pmm,
                    x_all, pmax, pmin, red8, s8, It,
                )
            else:
                _pass_b(
                    nc, mybir, Alu, psump, xsa, outa, pm64,
                    x_all, binm, mH, stag, s1v, gv8,
                    mnv, h4, At, Nt, ones1, sel_bias,
                )

    nc.compile()
    return nc


def _pass_a(
    nc, mybir, Alu, psump, xsa, pmm,
    x_all, pmax, pmin, red8, s8, It,
):
    """Load the shard and reduce it to [mx(4) | -mn(4)] -> DRAM pmm[8,1]."""
    f32 = mybir.dt.float32
    for c in range(C):
        nc.sync.dma_start(out=x_all[:, c, :, :], in_=xsa[:, c, :, :, :])
    for c in range(C):
        for k in range(4):
            chunk = x_all[:, c, 1 + 8 * k : 9 + 8 * k, :]
            nc.vector.tensor_reduce(
                out=pmax[:, 4 * c + k : 4 * c + k + 1],
                in_=chunk,
                axis=mybir.AxisListType.XY,
                op=Alu.max,
            )
            nc.vector.tensor_reduce(
                out=pmin[:, 4 * c + k : 4 * c + k + 1],
                in_=chunk,
                axis=mybir.AxisListType.XY,
                op=Alu.min,
            )
    for c in range(C):
        nc.vector.tensor_reduce(
            out=red8[:, c : c + 1],
            in_=pmax[:, 4 * c : 4 * c + 4],
            axis=mybir.AxisListType.X,
            op=Alu.max,
        )
        nc.vector.tensor_reduce(
            out=red8[:, 4 + c : 5 + c],
            in_=pmin[:, 4 * c : 4 * c + 4],
            axis=mybir.AxisListType.X,
            op=Alu.min,
        )
    # negate the mins so a single max combines both downstream
    nc.vector.tensor_scalar_mul(red8[:, 4:8], red8[:, 4:8], -1.0)
    # cross-partition max: transpose red8 [128p, 8] -> psum [8p, 128] with a
    # PE matmul against the identity, then free-axis reduce on DVE.
    pst = psump.tile([128, 2048], f32, tag="ps")
    nc.tensor.matmul(pst[0:8, 0:128], red8[:, :], It[:, :], start=True, stop=True)
    nc.vector.tensor_reduce(
        out=s8[0:8, 0:1],
        in_=pst[0:8, 0:128],
        axis=mybir.AxisListType.X,
        op=Alu.max,
    )
    nc.sync.dma_start(out=pmm.ap(), in_=s8[0:8, 0:1])


def _pass_b(
    nc, mybir, Alu, psump, xsa, outa, pm64,
    x_all, binm, mH, stag, s1v, gv8,
    mnv, h4, At, Nt, ones1, sel_bias,
):
    """Main pipeline: thresholds from pm64, mask, dilate, boundary."""
    f32 = mybir.dt.float32
    for c in range(C):
        nc.sync.dma_start(out=x_all[:, c, :, :], in_=xsa[:, c, :, :, :])

    # reduce the gathered per-core partials [8 cores, 8] over cores on
    # partition 0, then broadcast to all partitions with a rank-1 matmul.
    nc.sync.dma_start(out=s1v[0:1, 0:64], in_=pm64.ap().rearrange("k j -> (k j)")[None, :])
    nc.vector.tensor_reduce(
        out=s1v[0:1, 64:72],
        in_=s1v[0:1, 0:64].rearrange("p (k j) -> p j k", k=NCORES),
        axis=mybir.AxisListType.X,
        op=Alu.max,
    )
    psb = psump.tile([128, 2048], f32, tag="ps")
    nc.tensor.matmul(psb[:, 0:8], ones1[0:1, :], s1v[0:1, 64:72], start=True, stop=True)
    nc.vector.tensor_copy(gv8[:, :], psb[:, 0:8])
    nc.vector.tensor_scalar_mul(mnv[:, :], gv8[:, 4:8], -1.0)
    nc.vector.tensor_add(h4[:, :], gv8[:, 0:4], gv8[:, 4:8])
    nc.vector.tensor_scalar_mul(h4[:, :], h4[:, :], 0.5)

    # ---- mask, dilate, boundary ----
    mHd = mH[:, MHW : MHW + 32 * MHW].rearrange("p (r z) -> p r z", z=MHW)[
        :, :, 0:W
    ]
    for c in range(C):
        nc.vector.tensor_scalar(
            out=binm[:, :, :],
            in0=x_all[:, c, :, :],
            scalar1=mnv[:, c : c + 1],
            scalar2=h4[:, c : c + 1],
            op0=Alu.subtract,
            op1=Alu.is_gt,
        )
        nc.vector.tensor_tensor(
            out=mHd,
            in0=binm[:, 0:HS, :],
            in1=binm[:, 2 : HS + 2, :],
            op=Alu.max,
        )
        nc.vector.tensor_tensor(
            out=mHd,
            in0=mHd,
            in1=binm[:, 1 : HS + 1, :],
            op=Alu.max,
        )
        for t in range(2):  # 16 own rows per staging buffer
            ps = psump.tile([128, 2048], f32, tag="ps")
            ps2 = psump.tile([128, 2048], f32, tag="ps")
            for half, pst_ in ((0, ps), (1, ps2)):
                for s in range(4):  # one PSUM bank = 2 rows = 512
                    R = 16 * t + 8 * half + 2 * s
                    pslice = pst_[:, 512 * s : 512 * s + 512]
                    for j, dw in enumerate((-1, 0, 1)):
                        off = (R + 1) * MHW + dw
                        rhs = mH[:, off : off + 2 * MHW].rearrange(
                            "p (r z) -> p r z", z=MHW
                        )[:, :, 0:W]
                        nc.tensor.matmul(
                            pslice,
                            At[:, :],
                            rhs,
                            start=(j == 0),
                            stop=False,
                        )
                    nc.tensor.matmul(
                        pslice,
                        Nt[:, :],
                        binm[:, 1 + R : 3 + R, :],
                        start=False,
                        stop=True,
                    )
                nc.scalar.activation(
                    out=stag[:, 2048 * half : 2048 * half + 2048],
                    in_=pst_[:, :],
                    func=mybir.ActivationFunctionType.Sigmoid,
                    bias=sel_bias[:, :],
                    scale=200.0,
                )
            nc.sync.dma_start(
                out=outa[:, c, :, 16 * t : 16 * t + 16, :],
                in_=stag.rearrange("p (r w) -> p r w", w=W),
            )


def _get_nc(phase="B"):
    key = "nc" + phase
    if key not in _CACHE:
        _CACHE[key] = _build(phase=phase)
    return _CACHE[key]


def _make_in_maps(x: np.ndarray):
    in_maps = []
    for k in range(NCORES):
        xs = np.empty((B, C, D, HA, W), np.float32)
        lo = k * HS
        xs[:, :, :, 1 : HS + 1, :] = x[:, :, :, lo : lo + HS, :]
        if k > 0:
            xs[:, :, :, 0, :] = x[:, :, :, lo - 1, :]
        else:
            xs[:, :, :, 0, :] = HPAD
        if k < NCORES - 1:
            xs[:, :, :, HS + 1, :] = x[:, :, :, lo + HS, :]
        else:
            xs[:, :, :, HS + 1, :] = HPAD
        in_maps.append({"xs": xs})
    return in_maps


def kernel(x: np.ndarray) -> np.ndarray:
    from concourse.bass_utils import run_bass_kernel_spmd

    x = np.ascontiguousarray(np.asarray(x), dtype=np.float32)
    assert x.shape == (B, C, D, H, W)

    in_maps = _make_in_maps(x)
    cores = list(range(NCORES))

    # launch A: per-core min/max partials
    res_a = run_bass_kernel_spmd(_get_nc("A"), in_maps, core_ids=cores)
    pm64 = np.concatenate(
        [res_a.results[k]["pmm"].reshape(1, 8) for k in range(NCORES)], axis=0
    )

    # launch B: full pipeline with the gathered partials
    in_maps_b = [{"xs": m["xs"], "pm64": pm64} for m in in_maps]
    res = run_bass_kernel_spmd(_get_nc("B"), in_maps_b, core_ids=cores)
    pieces = [res.results[k]["out"] for k in range(NCORES)]
    return np.concatenate(pieces, axis=3)


if __name__ == "__main__":
    x = np.random.randn(B, C, D, H, W).astype(np.float32)
    y = kernel(x)
    print(y.shape, y.dtype, y.sum())
